# revision 73
# baseline (speedup 1.0000x reference)
"""Trainium2 Bass kernel for nn_CryptoGNN (2-layer GCN + pooled heads).

Math (same collapse as the validated baseline):
  With A = normalized adjacency (incl. self loops), P = [B,N] pooling,
  u[d] = sum_{s->d} dis[s]x[s],   zhat = (u + dis*x_self)@W1 + sqrt(deg)*b1,
  h1hat = relu(zhat);  true h1 = dis*h1hat, so the pooling matrix columns
  are pre-scaled by dis and layer 2 + heads collapse to tiny host math.

Per-core device pipeline (8-way node sharding, 12544 dst nodes/core,
7 dst-chunks so every stage is pipelined across engines):
  per chunk c:
    1. bf16 per-chunk compacted src table DMA ([128, TW_c]: 8 banks x 6
       feature rows; edges are round-robin balanced over banks, and only
       srcs referenced by this chunk's edges are present, so the gather
       is stream-bound; all 128 rows host-written - no SBUF garbage can
       reach the PE, 0*NaN != 0 there)
    2. expand bf16->fp32 split across Act/DVE (ap_gather needs 4B elems)
    3. GPSIMD ap_gather of the chunk's dst-sorted per-bank edge streams
    4. fp32 prefix scan (DVE), boundary ap_gather (GPSIMD), shifted
       diff -> dt bf16 (DVE)
    5. per 8-tile batch: z = dt_t^T @ selW + aug_t^T @ w1aug (bf16 PE;
       selW = bank-scattered W1 rows, aug = [dis*x_self; sqrt(deg)]),
       relu -> h1 bf16 (Act), G^T += h1_t^T @ papt_t into one [128,80]
       PSUM across all 98 tiles.
Host sums the 8 partial G^T and runs the small head in numpy.
"""

import sys

if "/opt/trn_rl_repo" not in sys.path:
    sys.path.insert(0, "/opt/trn_rl_repo")

import numpy as np
import ml_dtypes

N = 100000
E = 600000
B = 64
IN = 6
H = 128
S = 16

NG = 8                    # banks and cores
NS = 12544                # nodes per core shard (98*128)
NPAD = NS * NG            # 100352
NT = 98                   # node tiles per shard
# dst chunks per core, in node tiles; small first chunk ramps the pipeline
# up fast, descending tail shrinks the last diff->z->relu->G chain
TCH = (16, 16, 16, 16, 16, 8, 8, 2)
C = len(TCH)
NDCS = tuple(t * 128 for t in TCH)
# boundary widths == dst counts: the segmented scan yields per-segment sums
# directly, one gathered value per dst (empty cells gather slot 0 == 0.0)
NBCS = NDCS
DOFF = tuple(int(x) for x in np.concatenate([[0], np.cumsum(NDCS)]))
BOFF = DOFF
NBT = BOFF[-1]
PCOL = 80                 # papt columns: 64 PA + <=16 local P
P128 = 128

_compiled = {}


def _build_nc(TWS, JWS):
    import concourse.bacc as bacc
    import concourse.mybir as mybir
    from concourse import tile

    f32 = mybir.dt.float32
    bf16 = mybir.dt.bfloat16
    i16 = mybir.dt.int16

    TWT = sum(TWS)
    TOFF = np.concatenate([[0], np.cumsum(TWS)]).astype(int)
    JWT = sum(JWS)
    OFF = np.concatenate([[0], np.cumsum(JWS)]).astype(int)

    nc = bacc.Bacc("TRN2", target_bir_lowering=False, debug=False)

    xt = nc.declare_dram_parameter("xt", [P128, TWT], bf16, isOutput=False)
    gidx = nc.declare_dram_parameter("gidx", [P128, JWT // 16], i16, isOutput=False)
    bidx = nc.declare_dram_parameter("bidx", [P128, NBT // 16], i16, isOutput=False)
    mask = nc.declare_dram_parameter("mask", [P128, JWT], bf16, isOutput=False)
    aug = nc.declare_dram_parameter("aug", [7, NS], bf16, isOutput=False)
    selw = nc.declare_dram_parameter("selw", [P128, H], bf16, isOutput=False)
    w1aug = nc.declare_dram_parameter("w1aug", [7, H], bf16, isOutput=False)
    papt = nc.declare_dram_parameter("papt", [NT * P128, PCOL], bf16, isOutput=False)
    gout = nc.declare_dram_parameter("gout", [P128, PCOL], f32, isOutput=True)

    with tile.TileContext(nc) as tc:
        with (
            tc.tile_pool(name="big", bufs=1) as big,
            tc.tile_pool(name="small", bufs=1) as small,
            tc.tile_pool(name="tabp", bufs=3) as tabp,
            tc.tile_pool(name="maskp", bufs=2) as maskp,
            tc.tile_pool(name="bndp", bufs=1) as bndp,
            tc.tile_pool(name="hbuf", bufs=3) as hbuf,
            tc.tile_pool(name="psz", bufs=2, space="PSUM") as pszp,
            tc.tile_pool(name="psG", bufs=1, space="PSUM") as psGp,
        ):
            # preload the activation-function table while DMAs run
            warm = small.tile([1, 2], f32)
            nc.vector.memset(warm[:], 0.0)
            nc.scalar.activation(out=warm[:], in_=warm[:],
                                 func=mybir.ActivationFunctionType.Copy)

            xt_t = big.tile([P128, TWT], bf16, tag="xtb")
            gath = big.tile([P128, JWT], f32, tag="gath")
            dt = big.tile([P128, NS], bf16, tag="dt")
            TWMAX = max(int(w) for w in TWS)
            JWMAX = max(int(w) for w in JWS)
            tables = [None] * C
            masks = [None] * C

            gidx_t = small.tile([P128, JWT // 16], i16)
            bidx_t = small.tile([P128, NBT // 16], i16)
            aug_t = small.tile([7, NS], bf16)
            selw_t = small.tile([P128, H], bf16)
            w1_t = small.tile([7, H], bf16)
            papt_t = big.tile([P128, NT * PCOL], bf16, tag="papt")

            def dma_xt(c, half=None):
                t0, t1 = int(TOFF[c]), int(TOFF[c + 1])
                if half is not None:
                    mid = (t0 + t1) // 2 & ~1
                    t0, t1 = (t0, mid) if half == 0 else (mid, t1)
                nc.sync.dma_start(out=xt_t[:, t0:t1], in_=xt[:, t0:t1])

            def table_c(c):
                if tables[c] is None:
                    tables[c] = tabp.tile([P128, TWMAX], f32,
                                          tag=f"tab{c % 3}", name=f"table_{c}")
                return tables[c]

            def expand(c, engine, half=None):
                # Act takes the first 0.385 share, DVE (2x copy) the rest
                tab = table_c(c)
                x0 = int(TOFF[c])
                w0, w1 = 0, int(TWS[c])
                if half is not None:
                    mid = (w1 // 2) & ~1
                    w0, w1 = (0, mid) if half == 0 else (mid, w1)
                xa = (w0 + int((w1 - w0) * 0.385)) & ~1
                if engine == "act":
                    nc.scalar.activation(
                        out=tab[:, w0:xa], in_=xt_t[:, x0 + w0:x0 + xa],
                        func=mybir.ActivationFunctionType.Copy,
                    )
                else:
                    nc.vector.tensor_copy(out=tab[:, xa:w1],
                                          in_=xt_t[:, x0 + xa:x0 + w1])

            def g1_c(c):
                o0, o1 = int(OFF[c]), int(OFF[c + 1])
                nc.gpsimd.ap_gather(
                    out_ap=gath[:, o0:o1], in_ap=table_c(c)[:, 0:int(TWS[c])],
                    idxs_ap=gidx_t[:, o0 // 16:o1 // 16],
                    channels=P128, num_elems=int(TWS[c]), d=1,
                    num_idxs=int(JWS[c]),
                )

            def dma_mask(c):
                o0, o1 = int(OFF[c]), int(OFF[c + 1])
                masks[c] = maskp.tile([P128, JWMAX], bf16, tag=f"m{c % 2}",
                                      name=f"mask_{c}")
                nc.sync.dma_start(out=masks[c][:, 0:o1 - o0], in_=mask[:, o0:o1])

            def scan_c(c):
                # segmented scan: state = mask*state + v resets at each
                # segment start, so segment sums sit at the end slots
                o0, o1 = int(OFF[c]), int(OFF[c + 1])
                nc.vector.tensor_tensor_scan(
                    out=gath[:, o0:o1], data0=masks[c][:, 0:o1 - o0],
                    data1=gath[:, o0:o1],
                    initial=0.0, op0=mybir.AluOpType.mult,
                    op1=mybir.AluOpType.add,
                )

            bnds = [None] * C

            def g2_c(c):
                o0, o1 = int(OFF[c]), int(OFF[c + 1])
                bnd = bndp.tile([P128, NBCS[c]], f32, tag="bnd",
                                name=f"bnd_{c}")
                nc.gpsimd.ap_gather(
                    out_ap=bnd[:], in_ap=gath[:, o0:o1],
                    idxs_ap=bidx_t[:, BOFF[c] // 16:BOFF[c + 1] // 16],
                    channels=P128, num_elems=int(JWS[c]), d=1, num_idxs=NBCS[c],
                )
                bnds[c] = bnd

            def diff_c(c):
                # segmented scan already produced per-dst sums; just downcast
                d0, nd = DOFF[c], NDCS[c]
                nc.vector.tensor_copy(out=dt[:, d0:d0 + nd],
                                      in_=bnds[c][:, 0:nd])

            # ---------- issue order ----------
            # DMA: xt_0 (split halves for a faster start), idxs, xt_1.. consts
            dma_xt(0, half=0)
            dma_xt(0, half=1)
            nc.sync.dma_start(out=gidx_t[:], in_=gidx[:])
            dma_mask(0)
            dma_xt(1)
            nc.sync.dma_start(out=bidx_t[:], in_=bidx[:])
            dma_mask(1)
            for c in range(2, C):
                dma_xt(c)
                dma_mask(c)
            nc.sync.dma_start(out=aug_t[:], in_=aug[:])
            nc.sync.dma_start(out=selw_t[:], in_=selw[:])
            nc.sync.dma_start(out=w1_t[:], in_=w1aug[:])
            for c in range(C):
                nc.sync.dma_start(
                    out=papt_t[:, (DOFF[c] // 128) * PCOL:
                               (DOFF[c + 1] // 128) * PCOL].rearrange(
                        "p (u j) -> p u j", j=PCOL
                    ),
                    in_=papt[DOFF[c]:DOFF[c + 1], :].rearrange(
                        "(u p) j -> p u j", p=P128
                    ),
                )

            # Act: expands then relus (relu engine picked in batch loop)
            expand(0, "act", half=0)
            expand(0, "act", half=1)
            for c in range(1, C):
                expand(c, "act")

            # interleaved per-chunk pipeline.
            # Pool order: g1_0, g1_1, g2_0, g1_2, g2_1, ... g1_6, g2_5, g2_6
            # DVE order:  e0, e1, s0, e2, s1, d0, e3, s2, d1, ... (scans lead
            # diffs by one chunk; expands lead scans by two)
            expand(0, "dve", half=0)
            expand(0, "dve", half=1)
            expand(1, "dve")
            g1_c(0)
            scan_c(0)
            g1_c(1)
            for c in range(2, C):
                expand(c, "dve")
                scan_c(c - 1)
                g2_c(c - 2)
                g1_c(c)
                diff_c(c - 2)
            scan_c(C - 1)
            g2_c(C - 2)
            diff_c(C - 2)
            g2_c(C - 1)
            diff_c(C - 1)

            # ---------- phase B: z -> relu -> G (sw-pipelined batches) ----------
            G_ps = psGp.tile([P128, PCOL], f32, tag="G")
            QB = 8
            batches = []
            for c in range(C):
                t = DOFF[c] // 128
                left = TCH[c]
                while left > 0:
                    sz = min(QB, left)
                    batches.append((t, sz))
                    t += sz
                    left -= sz

            def z_mms(t0, m, ps):
                for u in range(m):
                    n0 = (t0 + u) * P128
                    nc.tensor.matmul(
                        out=ps[:, u * H:(u + 1) * H],
                        lhsT=dt[:, n0:n0 + P128], rhs=selw_t[:],
                        start=True, stop=False,
                    )
                    nc.tensor.matmul(
                        out=ps[:, u * H:(u + 1) * H],
                        lhsT=aug_t[:, n0:n0 + P128], rhs=w1_t[:],
                        start=False, stop=True,
                    )

            def g_mms(t0, m, h1):
                for u in range(m):
                    t = t0 + u
                    nc.tensor.matmul(
                        out=G_ps[:],
                        lhsT=h1[:, u * H:(u + 1) * H],
                        rhs=papt_t[:, t * PCOL:(t + 1) * PCOL],
                        start=(t == 0), stop=(t == NT - 1),
                    )

            prev = None
            NBAT = len(batches)
            for bi, (t0, m) in enumerate(batches):
                ps = pszp.tile([P128, QB * H], f32, tag="z")
                z_mms(t0, m, ps)
                h1 = hbuf.tile([P128, QB * H], bf16, tag="h1")
                if bi == NBAT - 2:
                    # DVE is free once the last diff retires; split this relu
                    # across Act and DVE so the tail batch finishes sooner
                    hm = (m // 2) * H
                    nc.scalar.activation(
                        out=h1[:, :hm], in_=ps[:, :hm],
                        func=mybir.ActivationFunctionType.Relu,
                    )
                    nc.vector.tensor_scalar_max(
                        out=h1[:, hm:m * H], in0=ps[:, hm:m * H], scalar1=0.0,
                    )
                else:
                    nc.scalar.activation(
                        out=h1[:, :m * H], in_=ps[:, :m * H],
                        func=mybir.ActivationFunctionType.Relu,
                    )
                if prev is not None:
                    g_mms(*prev)
                prev = (t0, m, h1)
            g_mms(*prev)

            G_sb = small.tile([P128, PCOL], f32)
            nc.vector.tensor_copy(out=G_sb[:], in_=G_ps[:])
            nc.sync.dma_start(out=gout[:], in_=G_sb[:])

    nc.compile()
    return nc


def _preprocess(x, edge_index, batch_idx):
    """Integer/structure preprocessing -> per-core device inputs."""
    src = np.asarray(edge_index[0], dtype=np.int64)
    dst = np.asarray(edge_index[1], dtype=np.int64)

    deg = (np.bincount(dst, minlength=N) + 1).astype(np.float32)
    dis = (1.0 / np.sqrt(deg)).astype(np.float32)
    sq = np.sqrt(deg).astype(np.float32)
    dis_pad = np.zeros(NPAD, np.float32)
    dis_pad[:N] = dis
    sq_pad = np.zeros(NPAD, np.float32)
    sq_pad[:N] = sq

    bi = np.asarray(batch_idx, dtype=np.int64)
    cnt = np.bincount(bi, minlength=B).astype(np.float32)

    x_np = np.asarray(x, dtype=np.float32)
    x_pad = np.zeros((NPAD, IN), np.float32)
    x_pad[:N] = x_np
    disx = x_pad * dis_pad[:, None]          # [NPAD, 6]

    # ---- pooling matrices (dense PA = P @ A) ----
    loop = np.arange(N, dtype=np.int64)
    src2 = np.concatenate([src, loop])
    dst2 = np.concatenate([dst, loop])
    w = (dis[src2] * dis[dst2]).astype(np.float64)
    flat = bi[dst2] * NPAD + src2
    PA = np.bincount(flat, weights=w, minlength=B * NPAD).reshape(B, NPAD)
    PA = PA.astype(np.float32)
    Pm = np.zeros((B, NPAD), np.float32)
    Pm[bi, np.arange(N)] = 1.0
    papt_full = (np.concatenate([PA, Pm], axis=0) * dis_pad[None, :]).T  # [NPAD,128]

    # graph span per core (for the P columns)
    first_graph = np.zeros(NG, np.int64)
    span = np.zeros(NG, np.int64)
    for k in range(NG):
        lo, hi = k * NS, min((k + 1) * NS, N)
        if lo >= N:
            first_graph[k] = B - 1
            span[k] = 1
            continue
        gset = bi[lo:hi]
        first_graph[k] = gset[0]
        span[k] = gset[-1] - gset[0] + 1
        assert span[k] <= PCOL - B, f"graph span {span[k]} > {PCOL - B}"

    # ---- per (core, chunk) streams, edges round-robin balanced on banks ----
    core = dst // NS
    dst_local = dst - core * NS
    chunk = np.searchsorted(np.asarray(DOFF[1:]), dst_local, side="right")
    # sort by (core, chunk, dst_local); bank = rank within group mod NG
    key0 = (core * C + chunk) * NS + dst_local
    order0 = np.argsort(key0, kind="stable")
    grp = (core * C + chunk)[order0]
    gstarts = np.searchsorted(grp, np.arange(NG * C))
    rank = np.arange(E) - np.concatenate(
        [[0], np.cumsum(np.bincount(grp, minlength=NG * C))])[grp]
    bank_e = np.empty(E, np.int64)
    bank_e[order0] = rank % NG

    # final order: (core, chunk, bank, dst_local)
    key = (((core * C + chunk) * NG + bank_e)) * NS + dst_local
    order = np.argsort(key, kind="stable")
    src_s = src[order]
    dstl_s = dst_local[order]

    cell = ((core * C + chunk) * NG + bank_e)[order]
    cellcnt = np.bincount(cell, minlength=NG * C * NG)
    cell_starts = np.zeros(NG * C * NG + 1, np.int64)
    np.cumsum(cellcnt, out=cell_starts[1:])
    cc = cellcnt.reshape(NG, C, NG)

    # per-chunk stream widths (+1 lead pad, pad to 32 for idx alignment)
    JWS = []
    for c in range(C):
        m = int(cc[:, c, :].max())
        JWS.append(((m + 1 + 31) // 32) * 32)
    JWT = sum(JWS)
    OFF = np.concatenate([[0], np.cumsum(JWS)]).astype(int)

    # per-chunk compacted tables
    colmaps = {}
    ncols = np.zeros((NG, C, NG), np.int64)
    for k in range(NG):
        for c in range(C):
            for g in range(NG):
                ci = (k * C + c) * NG + g
                s0, s1 = cell_starts[ci], cell_starts[ci + 1]
                uniq = np.unique(src_s[s0:s1])
                colmaps[(k, c, g)] = uniq
                ncols[k, c, g] = len(uniq)
    TWS = []
    for c in range(C):
        t = int(ncols[:, c, :].max()) + 1
        TWS.append((t + 15) & ~15)
    TWT = sum(TWS)
    TOFF = np.concatenate([[0], np.cumsum(TWS)]).astype(int)

    xt_all = np.zeros((NG, P128, TWT), ml_dtypes.bfloat16)
    gidx_all = np.zeros((NG, P128, JWT // 16), np.int16)
    bidx_all = np.zeros((NG, P128, NBT // 16), np.int16)
    mask_all = np.ones((NG, P128, JWT), ml_dtypes.bfloat16)

    disx_bf = disx.astype(ml_dtypes.bfloat16)
    for k in range(NG):
        for c in range(C):
            t0 = int(TOFF[c])
            for g in range(NG):
                uniq = colmaps[(k, c, g)]
                xt_all[k, 16 * g:16 * g + 6, t0 + 1:t0 + 1 + len(uniq)] = (
                    disx_bf[uniq].T
                )
                ci = (k * C + c) * NG + g
                s0, s1 = cell_starts[ci], cell_starts[ci + 1]
                ncell = s1 - s0
                comp = np.searchsorted(uniq, src_s[s0:s1]) + 1
                stream = np.zeros(JWS[c], np.int64)
                stream[1:1 + ncell] = comp
                blk = stream.reshape(JWS[c] // 16, 16).T.astype(np.int16)
                gidx_all[k, 16 * g:16 * (g + 1),
                         OFF[c] // 16:OFF[c + 1] // 16] = blk

                nd = NDCS[c]
                dloc = dstl_s[s0:s1] - DOFF[c]
                cnts = np.bincount(dloc, minlength=nd)
                ends = np.cumsum(cnts)
                # mask = 0 at each segment's first slot (and slot 0)
                nz = cnts > 0
                starts = 1 + ends[nz] - cnts[nz]
                mrow = np.ones(JWS[c], ml_dtypes.bfloat16)
                mrow[0] = 0
                mrow[starts] = 0
                mask_all[k, 16 * g:16 * (g + 1),
                         OFF[c]:OFF[c + 1]] = mrow[None, :]
                # gather the segment-end slot per dst; empty -> slot 0 (== 0.0)
                blist = np.where(nz, ends, 0)
                bblk = blist.reshape(nd // 16, 16).T.astype(np.int16)
                bidx_all[k, 16 * g:16 * (g + 1),
                         BOFF[c] // 16:BOFF[c + 1] // 16] = bblk

    # aug rows: 0-5 dis*x own chunk (self loop), 6 sqrt(deg) (carries b1)
    aug_all = np.zeros((NG, 7, NS), ml_dtypes.bfloat16)
    for k in range(NG):
        n0 = k * NS
        aug_all[k, 0:6] = disx[n0:n0 + NS].T.astype(ml_dtypes.bfloat16)
        aug_all[k, 6] = sq_pad[n0:n0 + NS].astype(ml_dtypes.bfloat16)

    # papt per core: 64 PA cols + local P cols, blocked [NT*128, PCOL]
    papt_all = np.zeros((NG, NT * P128, PCOL), ml_dtypes.bfloat16)
    for k in range(NG):
        n0 = k * NS
        pk = np.zeros((NS, PCOL), np.float32)
        pk[:, :B] = papt_full[n0:n0 + NS, :B]
        b0, sp = first_graph[k], span[k]
        pk[:, B:B + sp] = papt_full[n0:n0 + NS, B + b0:B + b0 + sp]
        papt_all[k] = pk.astype(ml_dtypes.bfloat16)

    return {
        "JW": (tuple(TWS), tuple(JWS)),
        "TWS": TWS,
        "JWS": JWS,
        "xt_all": xt_all,
        "gidx_all": gidx_all,
        "bidx_all": bidx_all,
        "mask_all": mask_all,
        "aug_all": aug_all,
        "papt_all": papt_all,
        "first_graph": first_graph,
        "span": span,
        "cnt": cnt,
    }


def _head(G, cnt, inputs):
    f = np.float32
    W2 = np.asarray(inputs["W2"], f)
    b2 = np.asarray(inputs["b2"], f)
    Wg = np.asarray(inputs["Wg"], f)
    bg = np.asarray(inputs["bg"], f)
    Et = np.asarray(inputs["Et"], f)
    Ek = np.asarray(inputs["Ek"], f)
    Ev = np.asarray(inputs["Ev"], f)
    Wp = np.asarray(inputs["Wp"], f)
    bp = np.asarray(inputs["bp"], f)
    Ekid = np.asarray(inputs["Ekid"], f)
    Wc = np.asarray(inputs["Wc"], f)
    bc = np.asarray(inputs["bc"], f)
    Wl = np.asarray(inputs["Wl"], f)
    bl = np.asarray(inputs["bl"], f)
    Wm1 = np.asarray(inputs["Wm1"], f)
    bm1 = np.asarray(inputs["bm1"], f)
    Wm2 = np.asarray(inputs["Wm2"], f)
    bm2 = np.asarray(inputs["bm2"], f)
    st = np.asarray(inputs["sol_type_idx"], np.int64)
    sk = np.asarray(inputs["sol_key_idx"], np.int64)
    sv = np.asarray(inputs["sol_val_idx"], np.int64)
    kid = np.asarray(inputs["kernel_id"], np.int64)
    cond = np.asarray(inputs["cond_vec"], f)
    loc = np.asarray(inputs["local_feats"], f)

    relu = lambda a: np.maximum(a, 0.0).astype(f)

    Ph2 = G[:B] @ W2 + cnt[:, None] * b2[None, :] + G[B:]
    g = (Ph2 / np.maximum(cnt, 1.0)[:, None]) @ Wg + bg

    seq_mean = np.concatenate(
        [Et[st].mean(axis=1), Ek[sk].mean(axis=1), Ev[sv].mean(axis=1)], axis=-1
    ).astype(f)
    p = relu(seq_mean @ Wp + bp)
    kvec = Ekid[kid]
    c = relu(cond @ Wc + bc)
    l = relu(loc @ Wl + bl)
    xf = np.concatenate([g, p, kvec, c, l], axis=1).astype(f)
    return (relu(xf @ Wm1 + bm1) @ Wm2 + bm2).astype(f)


def kernel(**inputs) -> np.ndarray:
    from concourse.bass_utils import run_bass_kernel_spmd

    pre = _preprocess(inputs["x"], inputs["edge_index"], inputs["batch_idx"])
    sig = pre["JW"]
    if sig not in _compiled:
        _compiled[sig] = _build_nc(tuple(pre["TWS"]), tuple(pre["JWS"]))
    nc = _compiled[sig]

    W1 = np.asarray(inputs["W1"], np.float32)
    b1 = np.asarray(inputs["b1"], np.float32)
    w1aug = np.concatenate([W1, b1[None, :]], axis=0).astype(ml_dtypes.bfloat16)
    selw = np.zeros((P128, H), ml_dtypes.bfloat16)
    for g in range(NG):
        selw[16 * g:16 * g + 6] = W1.astype(ml_dtypes.bfloat16)

    in_maps = []
    for k in range(NG):
        in_maps.append({
            "xt": pre["xt_all"][k],
            "gidx": pre["gidx_all"][k],
            "bidx": pre["bidx_all"][k],
            "mask": pre["mask_all"][k],
            "aug": pre["aug_all"][k],
            "selw": selw,
            "w1aug": w1aug,
            "papt": pre["papt_all"][k],
        })

    res = run_bass_kernel_spmd(nc, in_maps, core_ids=list(range(NG)))

    Gpa = np.zeros((B, H), np.float64)
    Gp = np.zeros((B, H), np.float64)
    for k, r in enumerate(res.results):
        gt = r["gout"].astype(np.float64)      # [128 f, 80 c]
        Gpa += gt[:, :B].T
        b0, sp = pre["first_graph"][k], pre["span"][k]
        Gp[b0:b0 + sp] += gt[:, B:B + sp].T
    G = np.concatenate([Gpa, Gp], axis=0).astype(np.float32)   # [128, H]

    return _head(G, pre["cnt"], inputs)
